# revision 2
# baseline (speedup 1.0000x reference)
"""ConvNAT (conv QKV + 2D dilated neighborhood attention) on 8 trn2 cores.

Sharding: core = (batch b, H-slab of 24 rows).  Each core computes conv
q/k/v for its slab (+12-row halo recompute) and the attention for its 24
output rows.  H-border rows (h<6, h>=90), whose NATTEN windows are clamped
and would break SPMD program uniformity, are computed on the host in numpy
and overwrite the device output.

Attention on device (per output row h):
  logits(96q x 7kr x 96kc) = Q_ext^T K_ext   (f32r matmuls, N>=256)
    Q_ext/K_ext = [conv channels (64) | 7 "h-distance" channels that
    reproduce scale*pe_h.pe_h(h-h') exactly for |h-h'|<=6 via a cosine
    interpolation], q side pre-scaled by 128^-0.5.
  DVE adds WBIAS[w,kc] = scale*pe_w.pe_w + (0 / -30000) W-band+parity mask.
  ACT exp -> P (bf16) with accum_out giving the softmax denominator free.
  PE transposes P per key row; AV = sum_i P_i^T V_i^T (bf16) into psum;
  DVE multiplies by 1/den during psum evacuation.
"""
import os
import re
import sys

sys.path.insert(0, '/opt/trn_rl_repo')

import numpy as np

import concourse.bass as bass
from concourse import mybir
from concourse.tile import TileContext
from concourse.masks import make_identity
from bass_rust import ScopedClock, VectorClock

F32 = mybir.dt.float32
F32R = mybir.dt.float32r
BF16 = mybir.dt.bfloat16

B, CIN, H, W = 2, 64, 96, 96
CI, CO = 64, 128
KS, DIL = 7, 2
SCALE = float(CI * 2) ** -0.5  # Cqk = 128 after pe concat
HS = 24          # rows per core
NH = 4           # h-slabs
NCORES = 8
NDIST = 7        # h-distance channels
CH = 64 + NDIST  # contraction channels
KV = 48          # k/v rows per core (24 + 12 halo each side, unclamped)
XR = 50          # x slab rows (KV + conv halo)
NEG = -30000.0

# ---------------------------------------------------------------- compat ---
MAX_WAITS = 1


def _patched_drain(self, tick_clock, wait_clock):
    nc = self.nc
    ticks = [int(v) for v in re.findall(r'\d+', repr(tick_clock.global_clock))]
    for i in range(0, len(ticks), MAX_WAITS):
        chunk = [0] * len(ticks)
        chunk[i:i + MAX_WAITS] = ticks[i:i + MAX_WAITS]
        if any(chunk):
            probe = nc.sync.nop()
            wait_clock.add_sem_waits(probe.ins, ScopedClock({None: VectorClock(chunk)}))
    nc.sync.drain()
    nc.all_engine_barrier()
    popped = nc._tile_sem_poison_stack.pop()
    assert popped is self._sem_poison
    nc.clear_and_free_semaphores(list(self.sems.allocated().values()))
    nc.all_engine_barrier()


TileContext._drain_and_barrier = _patched_drain


def _split_excess_waits(nc, max_waits=MAX_WAITS):
    n_split = 0
    for fn in nc.m.functions:
        for bb in fn.blocks:
            out = []
            changed = False
            for inst in bb.instructions:
                si = inst.sync_info
                waits = list(si.on_wait) if si and si.on_wait else []
                if len(waits) > max_waits:
                    extra = waits[:-max_waits]
                    for j in range(0, len(extra), max_waits):
                        nop = mybir.InstNoOp(name=f"{inst.name}-ws{j}", ins=[], outs=[])
                        nop.engine = inst.engine
                        nop.sync_info = mybir.SyncInfo(
                            on_wait=extra[j:j + max_waits], on_update=[])
                        out.append(nop)
                    si.on_wait = waits[-max_waits:]
                    changed = True
                    n_split += 1
                out.append(inst)
            if changed:
                bb.instructions = out
    return n_split


# ------------------------------------------------------------- host math ---
def _sincos(length, dim):
    half = dim // 2
    inv_freq = 1.0 / (10000.0 ** (np.arange(half, dtype=np.float64) * 2.0 / dim))
    ang = np.arange(length, dtype=np.float64)[:, None] * inv_freq[None, :]
    return np.concatenate([np.sin(ang), np.cos(ang)], axis=-1)  # (L, dim)


def _na_indices(L, K, D):
    i = np.arange(L)
    g = i % D
    r = i // D
    Lg = (L - g + D - 1) // D
    start = np.clip(r - K // 2, 0, Lg - K)
    return g[:, None] + (start[:, None] + np.arange(K)[None, :]) * D  # (L, K)


def _hdist_channels():
    """QD,KD (NDIST, 96): sum_m QD[m,h]*KD[m,h'] == SCALE*pe_h[h].pe_h[h']
    exactly for even |h-h'| <= 6."""
    pe = _sincos(H, 32)           # (96, 32)
    inv_freq = 1.0 / (10000.0 ** (np.arange(16, dtype=np.float64) * 2.0 / 32))
    dv = np.array([0., 2., 4., 6.])
    g = SCALE * np.cos(dv[:, None] * inv_freq[None, :]).sum(1)  # exact pe.pe(d)
    th = np.arange(4, dtype=np.float64) * (np.pi / 6.0)
    M = np.cos(dv[:, None] * th[None, :])                        # (4, 4)
    b = np.linalg.solve(M, g)
    hh = np.arange(H, dtype=np.float64)
    QD = np.zeros((NDIST, H))
    KD = np.zeros((NDIST, H))
    QD[0] = b[0]
    KD[0] = 1.0
    for m in range(1, 4):
        QD[2 * m - 1] = b[m] * np.cos(th[m] * hh)
        QD[2 * m] = b[m] * np.sin(th[m] * hh)
        KD[2 * m - 1] = np.cos(th[m] * hh)
        KD[2 * m] = np.sin(th[m] * hh)
    # verify
    got = QD.T @ KD
    ref = SCALE * (pe @ pe.T)
    for d in (-6, -4, -2, 0, 2, 4, 6):
        idx = np.arange(max(0, -d), min(H, H - d))
        err = np.abs(got[idx, idx + d] - ref[idx, idx + d]).max()
        assert err < 1e-6, (d, err)
    return QD.astype(np.float32), KD.astype(np.float32)


def _wbias():
    pe = _sincos(W, 32)
    idx_w = _na_indices(W, KS, DIL)   # (96, 7)
    wb = np.full((W, W), NEG, dtype=np.float64)
    dot = SCALE * (pe @ pe.T)
    for w in range(W):
        wb[w, idx_w[w]] = dot[w, idx_w[w]]
    return wb.astype(np.float32)


def _conv_np(x, w, bias, rows):
    """NCHW 3x3 pad-1 conv evaluated at `rows` -> (B, len(rows), 96, Cout)."""
    Bn, Cin, Hn, Wn = x.shape
    xp = np.zeros((Bn, Cin, Hn + 2, Wn + 2), dtype=np.float64)
    xp[:, :, 1:-1, 1:-1] = x
    rows = np.asarray(rows)
    acc = np.zeros((Bn, len(rows), Wn, w.shape[0]), dtype=np.float64)
    for ky in range(3):
        for kx in range(3):
            xs = xp[:, :, rows + ky, :][:, :, :, kx:kx + Wn]  # (B,C,R,W)
            acc += np.einsum('bcrw,oc->brwo', xs, w[:, :, ky, kx].astype(np.float64))
    return acc + bias[None, None, None, :].astype(np.float64)


def _host_border(x, wq, bq, wk, bk, wv, bv):
    """Reference computation for the clamped border rows. -> dict h -> (B,96,128)."""
    border_h = list(range(0, 6)) + list(range(90, 96))
    kv_rows = sorted(set(np.concatenate([_na_indices(H, KS, DIL)[h] for h in border_h])))
    kv_rows = np.asarray(kv_rows)
    q_c = _conv_np(x, wq, bq, np.asarray(border_h))     # (B, 12, 96, 64)
    k_c = _conv_np(x, wk, bk, kv_rows)                  # (B, R, 96, 64)
    v_c = _conv_np(x, wv, bv, kv_rows)                  # (B, R, 96, 128)
    kv_pos = {r: i for i, r in enumerate(kv_rows)}
    pe_h = _sincos(H, 32)
    pe_w = _sincos(W, 32)
    idx_h = _na_indices(H, KS, DIL)
    idx_w = _na_indices(W, KS, DIL)
    out = {}
    for bi, h in enumerate(border_h):
        pe_q = np.concatenate([np.repeat(pe_h[h][None], W, 0), pe_w], axis=1)  # (96,64)
        q = np.concatenate([q_c[:, bi], np.repeat(pe_q[None], B, 0)], axis=2)  # (B,96,128)
        rows = [kv_pos[r] for r in idx_h[h]]
        kk = k_c[:, rows]                                   # (B,7,96,64)
        vv = v_c[:, rows]                                   # (B,7,96,128)
        pe_k = np.concatenate(
            [np.repeat(pe_h[idx_h[h]][:, None, :], W, 1),
             np.repeat(pe_w[None], KS, 0)], axis=2)         # (7,96,64)
        kk = np.concatenate([kk, np.repeat(pe_k[None], B, 0)], axis=3)  # (B,7,96,128)
        kn = kk[:, :, idx_w]                                # (B,7,96,7,128)
        vn = vv[:, :, idx_w]
        logits = SCALE * np.einsum('bwc,biwjc->bwij', q, kn)   # (B,96,7,7)
        m = logits.reshape(B, W, -1).max(-1)
        p = np.exp(logits - m[:, :, None, None])
        p /= p.reshape(B, W, -1).sum(-1)[:, :, None, None]
        out[h] = np.einsum('bwij,biwjc->bwc', p, vn)        # (B,96,128)
    return out


# ------------------------------------------------------------ bass build ---
_CACHE = {}


def _build_program():
    if 'nc' in _CACHE:
        return _CACHE['nc']
    nc = bass.Bass('TRN2')
    xs = nc.dram_tensor('xs', (64, XR, 98), F32R, kind='ExternalInput')
    wqk2 = nc.dram_tensor('wqk2', (3, 128, 128), F32R, kind='ExternalInput')
    wqk1 = nc.dram_tensor('wqk1', (3, 64, 128), F32R, kind='ExternalInput')
    wv2 = nc.dram_tensor('wv2', (3, 128, 128), F32R, kind='ExternalInput')
    wv1 = nc.dram_tensor('wv1', (3, 64, 128), F32R, kind='ExternalInput')
    qkb = nc.dram_tensor('qkb', (128, 1), F32, kind='ExternalInput')
    vb = nc.dram_tensor('vb', (128, 1), F32, kind='ExternalInput')
    qd = nc.dram_tensor('qd', (NDIST, HS * 96), F32R, kind='ExternalInput')
    kd = nc.dram_tensor('kd', (NDIST, KV * 96), F32R, kind='ExternalInput')
    wbt = nc.dram_tensor('wbt', (96, 96), F32, kind='ExternalInput')
    o = nc.dram_tensor('o', (HS, 96, 128), F32, kind='ExternalOutput')

    with TileContext(nc) as tc:
        with tc.tile_pool(name='persist', bufs=1) as pp:
            x2 = pp.tile([128, XR, 98], F32R)
            nc.sync.dma_start(out=x2[0:64], in_=xs[:])
            nc.sync.dma_start(out=x2[64:128, 0:XR - 1, :], in_=xs[:, 1:XR, :])
            w_qk2 = pp.tile([128, 3, 128], F32R)
            nc.sync.dma_start(out=w_qk2, in_=wqk2[:].rearrange('t p n -> p t n'))
            w_qk1 = pp.tile([64, 3, 128], F32R)
            nc.sync.dma_start(out=w_qk1, in_=wqk1[:].rearrange('t p n -> p t n'))
            w_v2 = pp.tile([128, 3, 128], F32R)
            nc.sync.dma_start(out=w_v2, in_=wv2[:].rearrange('t p n -> p t n'))
            w_v1 = pp.tile([64, 3, 128], F32R)
            nc.sync.dma_start(out=w_v1, in_=wv1[:].rearrange('t p n -> p t n'))
            qkbias = pp.tile([128, 1], F32)
            nc.sync.dma_start(out=qkbias, in_=qkb[:])
            vbias = pp.tile([128, 1], F32)
            nc.sync.dma_start(out=vbias, in_=vb[:])
            wb = pp.tile([96, 96], F32)
            nc.sync.dma_start(out=wb, in_=wbt[:])
            ident = pp.tile([128, 128], BF16)
            make_identity(nc, ident)

            q_ext = pp.tile([CH, HS * 96], F32R)
            k_ext = pp.tile([CH, KV * 96], F32R)
            vsb = pp.tile([128, KV * 96], BF16)
            vt = pp.tile([96, KV * 128], BF16)
            nc.sync.dma_start(out=q_ext[64:CH, :], in_=qd[:])
            nc.sync.dma_start(out=k_ext[64:CH, :], in_=kd[:])

            # ------------------------------------------------ convolution --
            # kv slab rows 0..47 = image rows h0-12 .. h0+35 (zero-padded x).
            # x slab row 0 = image h0-13; conv for kv row r uses slab rows
            # r, r+1, r+2 (ky=0,1,2).  Pair taps (ky=0,1) via the doubled-x
            # tile; ky=2 is a K=64 matmul on the low partitions.
            qoff = 12  # q rows are kv rows 12..35  (h0 .. h0+23)
            for half in range(2):
                for which in range(2):  # 0 = qk, 1 = v
                    with tc.tile_pool(name='cps', bufs=6, space='PSUM') as cps:
                        w2 = w_qk2 if which == 0 else w_v2
                        w1 = w_qk1 if which == 0 else w_v1
                        psl = []
                        for rg in range(6):
                            t_c = cps.tile([128, 4, 96], F32, tag='c')
                            psl.append(t_c)
                        for kx in range(3):
                            for rg in range(6):
                                r0 = half * 24 + rg * 4
                                nc.tensor.matmul(
                                    psl[rg][:], w2[:, kx, :],
                                    x2[:, r0:r0 + 4, kx:kx + 96],
                                    start=(kx == 0), stop=False)
                            for rg in range(6):
                                r0 = half * 24 + rg * 4
                                nc.tensor.matmul(
                                    psl[rg][:], w1[:, kx, :],
                                    x2[0:64, r0 + 2:r0 + 6, kx:kx + 96],
                                    start=False, stop=(kx == 2))
                        for rg in range(6):
                            r0 = half * 24 + rg * 4
                            if which == 0:
                                if qoff <= r0 < qoff + HS:
                                    nc.scalar.activation(
                                        out=q_ext[0:64, (r0 - qoff) * 96:(r0 - qoff + 4) * 96],
                                        in_=psl[rg][0:64], func=mybir.ActivationFunctionType.Identity,
                                        bias=qkbias[0:64])
                                nc.vector.tensor_scalar(
                                    out=k_ext[0:64, r0 * 96:(r0 + 4) * 96],
                                    in0=psl[rg][64:128], scalar1=qkbias[64:128],
                                    scalar2=None, op0=mybir.AluOpType.add)
                            else:
                                if rg % 2 == 0:
                                    nc.scalar.activation(
                                        out=vsb[:, r0 * 96:(r0 + 4) * 96],
                                        in_=psl[rg][:], func=mybir.ActivationFunctionType.Identity,
                                        bias=vbias[:])
                                else:
                                    nc.vector.tensor_scalar(
                                        out=vsb[:, r0 * 96:(r0 + 4) * 96],
                                        in0=psl[rg][:], scalar1=vbias[:],
                                        scalar2=None, op0=mybir.AluOpType.add)

            # ------------------------------------------------- V^T --------
            with tc.tile_pool(name='tps', bufs=2, space='PSUM') as tps:
                for grp in range(12):
                    pst = tps.tile([96, 4, 128], BF16, tag='t')
                    for rr in range(4):
                        r = grp * 4 + rr
                        nc.tensor.transpose(
                            pst[:, rr, :], vsb[:, r * 96:(r + 1) * 96], ident)
                    if grp % 2 == 0:
                        nc.scalar.copy(vt[:, grp * 512:(grp + 1) * 512], pst[:])
                    else:
                        nc.vector.tensor_copy(
                            out=vt[:, grp * 512:(grp + 1) * 512], in_=pst[:])

            # ---------------------------------------------- attention -----
            wb_ap = wb[:, :]
            wb_b = bass.AP(tensor=wb_ap.tensor, offset=wb_ap.offset,
                           ap=[wb_ap.ap[0], [0, KS], wb_ap.ap[1]])
            with tc.tile_pool(name='aps', bufs=2, space='PSUM') as aps, \
                 tc.tile_pool(name='tp2', bufs=1, space='PSUM') as tp2, \
                 tc.tile_pool(name='ops', bufs=2, space='PSUM') as ops, \
                 tc.tile_pool(name='att', bufs=2) as att:
                for j in range(HS):
                    kr0 = j + 6  # kv slab row of first key row (interior pattern)
                    psL = aps.tile([96, KS, 128], F32, tag='L')
                    lhs = q_ext[:, j * 96:(j + 1) * 96]
                    ke = k_ext[:].rearrange('c (r w) -> c r w', w=96)
                    nc.tensor.matmul(psL[:, 0:4, 0:96], lhs,
                                     ke[:, kr0:kr0 + 8:2, :], start=True, stop=True)
                    nc.tensor.matmul(psL[:, 4:7, 0:96], lhs,
                                     ke[:, kr0 + 8:kr0 + 14:2, :], start=True, stop=True)
                    lm = att.tile([96, KS, 96], F32, tag='lm')
                    nc.vector.tensor_tensor(out=lm[:], in0=psL[:, :, 0:96],
                                            in1=wb_b, op=mybir.AluOpType.add)
                    pexp = att.tile([96, KS, 96], BF16, tag='p')
                    den = att.tile([96, 1], F32, tag='den')
                    nc.scalar.activation(out=pexp[:], in_=lm[:],
                                         func=mybir.ActivationFunctionType.Exp,
                                         accum_out=den[:])
                    rden = att.tile([96, 1], F32, tag='rden')
                    nc.vector.reciprocal(out=rden[:], in_=den[:])
                    psT = tp2.tile([96, KS, 128], BF16, tag='T')
                    for i in range(KS):
                        nc.tensor.transpose(psT[:, i, 0:96], pexp[:, i, :],
                                            ident[0:96, 0:96])
                    pt = att.tile([96, KS, 96], BF16, tag='pt')
                    if j % 2 == 0:
                        nc.scalar.copy(pt[:], psT[:, :, 0:96])
                    else:
                        nc.vector.tensor_copy(out=pt[:], in_=psT[:, :, 0:96])
                    psO = ops.tile([96, 128], F32, tag='O')
                    for i in range(KS):
                        r = kr0 + 2 * i
                        nc.tensor.matmul(psO[:], pt[:, i, :],
                                         vt[:, r * 128:(r + 1) * 128],
                                         start=(i == 0), stop=(i == KS - 1))
                    oh = att.tile([96, 128], F32, tag='oh')
                    nc.vector.tensor_scalar_mul(oh[:], psO[:], rden[:])
                    nc.sync.dma_start(out=o[j], in_=oh[:])

    _split_excess_waits(nc)
    _CACHE['nc'] = nc
    return nc


# ---------------------------------------------------------------- kernel ---
def _make_in_maps(x, wq, bq, wk, bk, wv, bv):
    x = np.asarray(x, dtype=np.float32)
    wq = np.asarray(wq, dtype=np.float32)
    wk = np.asarray(wk, dtype=np.float32)
    wv = np.asarray(wv, dtype=np.float32)
    bq = np.asarray(bq, dtype=np.float32)
    bk = np.asarray(bk, dtype=np.float32)
    bv = np.asarray(bv, dtype=np.float32)
    QD, KD = _hdist_channels()
    wbias = _wbias()
    wq_s = wq * SCALE
    w2 = np.zeros((3, 128, 128), dtype=np.float32)
    w1 = np.zeros((3, 64, 128), dtype=np.float32)
    v2 = np.zeros((3, 128, 128), dtype=np.float32)
    v1 = np.zeros((3, 64, 128), dtype=np.float32)
    for kx in range(3):
        w2[kx, 0:64, 0:64] = wq_s[:, :, 0, kx].T
        w2[kx, 0:64, 64:128] = wk[:, :, 0, kx].T
        w2[kx, 64:128, 0:64] = wq_s[:, :, 1, kx].T
        w2[kx, 64:128, 64:128] = wk[:, :, 1, kx].T
        w1[kx, :, 0:64] = wq_s[:, :, 2, kx].T
        w1[kx, :, 64:128] = wk[:, :, 2, kx].T
        v2[kx, 0:64, :] = wv[:, :, 0, kx].T
        v2[kx, 64:128, :] = wv[:, :, 1, kx].T
        v1[kx, :, :] = wv[:, :, 2, kx].T
    qkbias = np.concatenate([bq * SCALE, bk]).reshape(128, 1).astype(np.float32)
    vbias = bv.reshape(128, 1).astype(np.float32)

    in_maps = []
    for core in range(NCORES):
        b, slab = core // NH, core % NH
        h0 = slab * HS
        xsl = np.zeros((64, XR, 98), dtype=np.float32)
        r_lo, r_hi = h0 - 13, h0 + 37  # image rows of slab
        src_lo, src_hi = max(0, r_lo), min(H, r_hi)
        xsl[:, src_lo - r_lo: src_hi - r_lo, 1:97] = x[b, :, src_lo:src_hi, :]
        qdf = np.repeat(QD[:, h0:h0 + HS, None], 96, axis=2).reshape(NDIST, -1)
        kdf = np.zeros((NDIST, KV, 96), dtype=np.float32)
        for r in range(KV):
            img = h0 - 12 + r
            kdf[:, r, :] = KD[:, img % H, None]  # out-of-range rows are masked
        in_maps.append({
            'xs': xsl, 'wqk2': w2, 'wqk1': w1, 'wv2': v2, 'wv1': v1,
            'qkb': qkbias, 'vb': vbias,
            'qd': np.ascontiguousarray(qdf, dtype=np.float32),
            'kd': np.ascontiguousarray(kdf.reshape(NDIST, -1)),
            'wbt': wbias,
        })
    return in_maps


def kernel(x, wq, bq, wk, bk, wv, bv):
    x = np.asarray(x, dtype=np.float32)
    wq = np.asarray(wq, dtype=np.float32)
    wk = np.asarray(wk, dtype=np.float32)
    wv = np.asarray(wv, dtype=np.float32)
    bq = np.asarray(bq, dtype=np.float32)
    bk = np.asarray(bk, dtype=np.float32)
    bv = np.asarray(bv, dtype=np.float32)

    nc = _build_program()
    in_maps = _make_in_maps(x=x, wq=wq, bq=bq, wk=wk, bk=bk, wv=wv, bv=bv)

    from concourse.bass_utils import run_bass_kernel_spmd
    res = run_bass_kernel_spmd(nc, in_maps, core_ids=list(range(NCORES)))
    globals()['_LAST_RES'] = res

    out = np.zeros((B, H, W, CO), dtype=np.float32)
    for core in range(NCORES):
        b, slab = core // NH, core % NH
        out[b, slab * HS:(slab + 1) * HS] = res.results[core]['o']

    border = _host_border(x, wq, bq, wk, bk, wv, bv)
    for h, val in border.items():
        out[:, h] = val.astype(np.float32)
    return out



# revision 7
# speedup vs baseline: 1.4444x; 1.4444x over previous
"""ConvNAT (conv QKV + 2D dilated neighborhood attention) on 8 trn2 cores.

Sharding: core = (batch b, H-slab of 24 rows).  Each core computes conv
q/k/v for its slab (+12-row halo recompute) and the attention for its 24
output rows.  H-border rows (h<6, h>=90), whose NATTEN windows are clamped
and would break SPMD program uniformity, are computed on the host in numpy
and overwrite the device output.

All matmul-path data is fp16 (psum accumulation fp32).  Conv packs the
ky=0,1 taps via a host-doubled x (row-shifted partition halves) and the
ky=2 kx=0,1 taps via a host-built column-shifted x copy: 5 matmuls per
4-row group instead of 6.

Attention on device (per output row h):
  logits(96q x 7kr x 96kc) = Q_ext^T K_ext   (fp16 matmuls)
    Q_ext/K_ext = [conv channels (64) | 7 "h-distance" channels that
    reproduce scale*pe_h.pe_h(h-h') exactly for |h-h'|<=6 via a cosine
    interpolation], q side pre-scaled by 128^-0.5.
  Pool engine adds WBIAS[w,kc] = scale*pe_w.pe_w + (0 / -30000) mask.
  ACT exp -> P (fp16) with accum_out collecting the softmax denominator
  into a persistent [96, 24] tile (DMAed out once; host divides).
  PE transposes P per key row; AV uses V^T slices as stationary and
  P^T as moving -> psum O^T[c, w] (unnormalized), evacuated + DMAed.
"""
import os
import re
import sys

sys.path.insert(0, '/opt/trn_rl_repo')

import numpy as np

import concourse.bass as bass
from concourse import mybir
from concourse.tile import TileContext
from concourse.masks import make_identity
from bass_rust import ScopedClock, VectorClock

F32 = mybir.dt.float32
F16 = mybir.dt.float16

B, CIN, H, W = 2, 64, 96, 96
CI, CO = 64, 128
KS, DIL = 7, 2
SCALE = float(CI * 2) ** -0.5  # Cqk = 128 after pe concat
HS = 24          # rows per core
NH = 4           # h-slabs
NCORES = 8
NDIST = 7        # h-distance channels
CH = 64 + NDIST  # contraction channels
KV = 48          # k/v rows per core (24 + 12 halo each side, unclamped)
XR = 50          # x slab rows (KV + conv halo)
NEG = -30000.0

# ---------------------------------------------------------------- compat ---
MAX_WAITS = 1


def _patched_drain(self, tick_clock, wait_clock):
    nc = self.nc
    ticks = [int(v) for v in re.findall(r'\d+', repr(tick_clock.global_clock))]
    for i in range(0, len(ticks), MAX_WAITS):
        chunk = [0] * len(ticks)
        chunk[i:i + MAX_WAITS] = ticks[i:i + MAX_WAITS]
        if any(chunk):
            probe = nc.sync.nop()
            wait_clock.add_sem_waits(probe.ins, ScopedClock({None: VectorClock(chunk)}))
    nc.sync.drain()
    nc.all_engine_barrier()
    popped = nc._tile_sem_poison_stack.pop()
    assert popped is self._sem_poison
    nc.clear_and_free_semaphores(list(self.sems.allocated().values()))
    nc.all_engine_barrier()


TileContext._drain_and_barrier = _patched_drain


def _split_excess_waits(nc, max_waits=MAX_WAITS):
    n_split = 0
    for fn in nc.m.functions:
        for bb in fn.blocks:
            out = []
            changed = False
            for inst in bb.instructions:
                si = inst.sync_info
                waits = list(si.on_wait) if si and si.on_wait else []
                if len(waits) > max_waits:
                    extra = waits[:-max_waits]
                    for j in range(0, len(extra), max_waits):
                        nop = mybir.InstNoOp(name=f"{inst.name}-ws{j}", ins=[], outs=[])
                        nop.engine = inst.engine
                        nop.sync_info = mybir.SyncInfo(
                            on_wait=extra[j:j + max_waits], on_update=[])
                        out.append(nop)
                    si.on_wait = waits[-max_waits:]
                    changed = True
                    n_split += 1
                out.append(inst)
            if changed:
                bb.instructions = out
    return n_split


# ------------------------------------------------------------- host math ---
def _sincos(length, dim):
    half = dim // 2
    inv_freq = 1.0 / (10000.0 ** (np.arange(half, dtype=np.float64) * 2.0 / dim))
    ang = np.arange(length, dtype=np.float64)[:, None] * inv_freq[None, :]
    return np.concatenate([np.sin(ang), np.cos(ang)], axis=-1)  # (L, dim)


def _na_indices(L, K, D):
    i = np.arange(L)
    g = i % D
    r = i // D
    Lg = (L - g + D - 1) // D
    start = np.clip(r - K // 2, 0, Lg - K)
    return g[:, None] + (start[:, None] + np.arange(K)[None, :]) * D  # (L, K)


def _hdist_channels():
    """QD,KD (NDIST, 96): sum_m QD[m,h]*KD[m,h'] == SCALE*pe_h[h].pe_h[h']
    exactly for even |h-h'| <= 6.  Magnitude-balanced per channel pair so
    fp16 rounding error stays small."""
    pe = _sincos(H, 32)           # (96, 32)
    inv_freq = 1.0 / (10000.0 ** (np.arange(16, dtype=np.float64) * 2.0 / 32))
    dv = np.array([0., 2., 4., 6.])
    g = SCALE * np.cos(dv[:, None] * inv_freq[None, :]).sum(1)  # exact pe.pe(d)
    th = np.arange(4, dtype=np.float64) * (np.pi / 6.0)
    M = np.cos(dv[:, None] * th[None, :])                        # (4, 4)
    b = np.linalg.solve(M, g)
    hh = np.arange(H, dtype=np.float64)
    QD = np.zeros((NDIST, H))
    KD = np.zeros((NDIST, H))
    QD[0] = b[0]
    KD[0] = 1.0
    for m in range(1, 4):
        QD[2 * m - 1] = b[m] * np.cos(th[m] * hh)
        QD[2 * m] = b[m] * np.sin(th[m] * hh)
        KD[2 * m - 1] = np.cos(th[m] * hh)
        KD[2 * m] = np.sin(th[m] * hh)
    # balance magnitudes channel-wise: |QD_m| ~ |KD_m|
    for m in range(NDIST):
        mq = np.abs(QD[m]).max()
        mk = np.abs(KD[m]).max()
        if mq > 0 and mk > 0:
            s = np.sqrt(mk / mq)
            QD[m] *= s
            KD[m] /= s
    # verify
    got = QD.T @ KD
    pe_ref = SCALE * (pe @ pe.T)
    for dd in (-6, -4, -2, 0, 2, 4, 6):
        idx = np.arange(max(0, -dd), min(H, H - dd))
        err = np.abs(got[idx, idx + dd] - pe_ref[idx, idx + dd]).max()
        assert err < 1e-6, (dd, err)
    return QD, KD


def _wbias():
    pe = _sincos(W, 32)
    idx_w = _na_indices(W, KS, DIL)   # (96, 7)
    wb = np.full((W, W), NEG, dtype=np.float64)
    dot = SCALE * (pe @ pe.T)
    for w in range(W):
        wb[w, idx_w[w]] = dot[w, idx_w[w]]
    return wb


def _conv_np(x, w, bias, rows):
    """NCHW 3x3 pad-1 conv evaluated at `rows` -> (B, len(rows), 96, Cout)."""
    Bn, Cin, Hn, Wn = x.shape
    xp = np.zeros((Bn, Cin, Hn + 2, Wn + 2), dtype=np.float64)
    xp[:, :, 1:-1, 1:-1] = x
    rows = np.asarray(rows)
    acc = np.zeros((Bn, len(rows), Wn, w.shape[0]), dtype=np.float64)
    for ky in range(3):
        for kx in range(3):
            xs = xp[:, :, rows + ky, :][:, :, :, kx:kx + Wn]  # (B,C,R,W)
            acc += np.einsum('bcrw,oc->brwo', xs, w[:, :, ky, kx].astype(np.float64))
    return acc + bias[None, None, None, :].astype(np.float64)


def _host_border(x, wq, bq, wk, bk, wv, bv):
    """Reference computation for the clamped border rows. -> dict h -> (B,96,128)."""
    border_h = list(range(0, 6)) + list(range(90, 96))
    kv_rows = sorted(set(np.concatenate([_na_indices(H, KS, DIL)[h] for h in border_h])))
    kv_rows = np.asarray(kv_rows)
    q_c = _conv_np(x, wq, bq, np.asarray(border_h))     # (B, 12, 96, 64)
    k_c = _conv_np(x, wk, bk, kv_rows)                  # (B, R, 96, 64)
    v_c = _conv_np(x, wv, bv, kv_rows)                  # (B, R, 96, 128)
    kv_pos = {r: i for i, r in enumerate(kv_rows)}
    pe_h = _sincos(H, 32)
    pe_w = _sincos(W, 32)
    idx_h = _na_indices(H, KS, DIL)
    idx_w = _na_indices(W, KS, DIL)
    out = {}
    for bi, h in enumerate(border_h):
        pe_q = np.concatenate([np.repeat(pe_h[h][None], W, 0), pe_w], axis=1)  # (96,64)
        q = np.concatenate([q_c[:, bi], np.repeat(pe_q[None], B, 0)], axis=2)  # (B,96,128)
        rows = [kv_pos[r] for r in idx_h[h]]
        kk = k_c[:, rows]                                   # (B,7,96,64)
        vv = v_c[:, rows]                                   # (B,7,96,128)
        pe_k = np.concatenate(
            [np.repeat(pe_h[idx_h[h]][:, None, :], W, 1),
             np.repeat(pe_w[None], KS, 0)], axis=2)         # (7,96,64)
        kk = np.concatenate([kk, np.repeat(pe_k[None], B, 0)], axis=3)  # (B,7,96,128)
        kn = kk[:, :, idx_w]                                # (B,7,96,7,128)
        vn = vv[:, :, idx_w]
        logits = SCALE * np.einsum('bwc,biwjc->bwij', q, kn)   # (B,96,7,7)
        m = logits.reshape(B, W, -1).max(-1)
        p = np.exp(logits - m[:, :, None, None])
        p /= p.reshape(B, W, -1).sum(-1)[:, :, None, None]
        out[h] = np.einsum('bwij,biwjc->bwc', p, vn)        # (B,96,128)
    return out


# ------------------------------------------------------------ bass build ---
_CACHE = {}


def _build_program():
    if 'nc' in _CACHE:
        return _CACHE['nc']
    nc = bass.Bass('TRN2')
    # x, row-doubled: [0:64]=rows, [64:128]=rows shifted +1 (for ky=0,1)
    xs = nc.dram_tensor('xs', (128, XR, 98), F16, kind='ExternalInput')
    # x, col-doubled: [0:64]=cols+0, [64:128]=cols+1 (for ky=2, kx=0,1)
    x3s = nc.dram_tensor('x3s', (128, XR, 97), F16, kind='ExternalInput')
    wqk2 = nc.dram_tensor('wqk2', (3, 128, 128), F16, kind='ExternalInput')
    wqka = nc.dram_tensor('wqka', (128, 128), F16, kind='ExternalInput')
    wqkc = nc.dram_tensor('wqkc', (64, 128), F16, kind='ExternalInput')
    wv2 = nc.dram_tensor('wv2', (3, 128, 128), F16, kind='ExternalInput')
    wva = nc.dram_tensor('wva', (128, 128), F16, kind='ExternalInput')
    wvc = nc.dram_tensor('wvc', (64, 128), F16, kind='ExternalInput')
    qkb = nc.dram_tensor('qkb', (128, 1), F32, kind='ExternalInput')
    vb = nc.dram_tensor('vb', (128, 1), F32, kind='ExternalInput')
    qd = nc.dram_tensor('qd', (NDIST, HS * 96), F16, kind='ExternalInput')
    kd = nc.dram_tensor('kd', (NDIST, KV * 96), F16, kind='ExternalInput')
    wbt = nc.dram_tensor('wbt', (96, 96), F16, kind='ExternalInput')
    o = nc.dram_tensor('o', (HS, 128, 96), F32, kind='ExternalOutput')
    dn = nc.dram_tensor('dn', (96, HS), F32, kind='ExternalOutput')

    with TileContext(nc) as tc:
        with tc.tile_pool(name='persist', bufs=1) as pp:
            # small inputs first so they are resident before conv starts
            w_qk2 = pp.tile([128, 3, 128], F16)
            nc.sync.dma_start(out=w_qk2, in_=wqk2[:].rearrange('t p n -> p t n'))
            w_qka = pp.tile([128, 128], F16)
            nc.sync.dma_start(out=w_qka, in_=wqka[:])
            w_qkc = pp.tile([64, 128], F16)
            nc.sync.dma_start(out=w_qkc, in_=wqkc[:])
            w_v2 = pp.tile([128, 3, 128], F16)
            nc.sync.dma_start(out=w_v2, in_=wv2[:].rearrange('t p n -> p t n'))
            w_va = pp.tile([128, 128], F16)
            nc.sync.dma_start(out=w_va, in_=wva[:])
            w_vc = pp.tile([64, 128], F16)
            nc.sync.dma_start(out=w_vc, in_=wvc[:])
            qkbias = pp.tile([128, 1], F32)
            nc.sync.dma_start(out=qkbias, in_=qkb[:])
            vbias = pp.tile([128, 1], F32)
            nc.sync.dma_start(out=vbias, in_=vb[:])
            wb = pp.tile([96, 96], F16)
            nc.sync.dma_start(out=wb, in_=wbt[:])
            ident = pp.tile([128, 128], F16)
            make_identity(nc, ident)

            q_ext = pp.tile([CH, HS * 96], F16)
            k_ext = pp.tile([CH, KV * 96], F16)
            vsb = pp.tile([128, KV * 96], F16)
            vt = pp.tile([96, KV * 128], F16)
            den_all = pp.tile([96, HS], F32)
            nc.sync.dma_start(out=q_ext[64:CH, :], in_=qd[:])
            nc.sync.dma_start(out=k_ext[64:CH, :], in_=kd[:])

            # x slabs, chunked so conv can start before the tail arrives
            x2 = pp.tile([128, XR, 98], F16)
            x3 = pp.tile([128, XR, 97], F16)
            chunks = [(0, 13), (13, 26), (26, 39), (39, XR)]
            for a, b_ in chunks:
                nc.sync.dma_start(out=x2[:, a:b_, :], in_=xs[:, a:b_, :])
                nc.sync.dma_start(out=x3[:, a:b_, :], in_=x3s[:, a:b_, :])

            # ------------------------------------------------ convolution --
            # kv slab rows 0..47 = image rows h0-12 .. h0+35 (zero-padded x).
            # x slab row 0 = image h0-13; conv for kv row r uses slab rows
            # r..r+2 (ky=0..2).  ky=0,1 pair via row-doubled x (3 matmuls,
            # kx=0..2); ky=2 kx=0,1 pair via col-doubled x3 (1 matmul);
            # ky=2 kx=2 is a K=64 matmul on the low partitions.
            qoff = 12  # q rows are kv rows 12..35  (h0 .. h0+23)
            with tc.tile_pool(name='cps', bufs=6, space='PSUM') as cps:
                for rg in range(12):
                    r0 = rg * 4
                    for which in range(2):  # 0 = qk, 1 = v
                        w2 = w_qk2 if which == 0 else w_v2
                        wa = w_qka if which == 0 else w_va
                        wc = w_qkc if which == 0 else w_vc
                        t_c = cps.tile([128, 4, 96], F32, tag='c')
                        for kx in range(3):
                            nc.tensor.matmul(
                                t_c[:], w2[:, kx, :],
                                x2[:, r0:r0 + 4, kx:kx + 96],
                                start=(kx == 0), stop=False)
                        nc.tensor.matmul(
                            t_c[:], wa, x3[:, r0 + 2:r0 + 6, 0:96],
                            start=False, stop=False)
                        nc.tensor.matmul(
                            t_c[:], w_qkc if which == 0 else w_vc,
                            x2[0:64, r0 + 2:r0 + 6, 2:98],
                            start=False, stop=True)
                        if which == 0:
                            if qoff <= r0 < qoff + HS:
                                nc.scalar.activation(
                                    out=q_ext[0:64, (r0 - qoff) * 96:(r0 - qoff + 4) * 96],
                                    in_=t_c[0:64], func=mybir.ActivationFunctionType.Identity,
                                    bias=qkbias[0:64])
                            nc.vector.tensor_scalar(
                                out=k_ext[0:64, r0 * 96:(r0 + 4) * 96],
                                in0=t_c[64:128], scalar1=qkbias[64:128],
                                scalar2=None, op0=mybir.AluOpType.add)
                        else:
                            if rg % 2 == 0:
                                nc.scalar.activation(
                                    out=vsb[:, r0 * 96:(r0 + 4) * 96],
                                    in_=t_c[:], func=mybir.ActivationFunctionType.Identity,
                                    bias=vbias[:])
                            else:
                                nc.vector.tensor_scalar(
                                    out=vsb[:, r0 * 96:(r0 + 4) * 96],
                                    in0=t_c[:], scalar1=vbias[:],
                                    scalar2=None, op0=mybir.AluOpType.add)

            # ------------------------------------------------- V^T --------
            with tc.tile_pool(name='tps', bufs=4, space='PSUM') as tps:
                for grp in range(12):
                    pst = tps.tile([96, 4, 128], F16, tag='t')
                    for rr in range(4):
                        r = grp * 4 + rr
                        nc.tensor.transpose(
                            pst[:, rr, :], vsb[:, r * 96:(r + 1) * 96], ident)
                    if grp % 2 == 0:
                        nc.scalar.copy(vt[:, grp * 512:(grp + 1) * 512], pst[:])
                    else:
                        nc.vector.tensor_copy(
                            out=vt[:, grp * 512:(grp + 1) * 512], in_=pst[:])

            # ---------------------------------------------- attention -----
            wb_ap = wb[:, :]
            wb_b = bass.AP(tensor=wb_ap.tensor, offset=wb_ap.offset,
                           ap=[wb_ap.ap[0], [0, KS], wb_ap.ap[1]])
            with tc.tile_pool(name='aps', bufs=2, space='PSUM') as aps, \
                 tc.tile_pool(name='tp2', bufs=1, space='PSUM') as tp2, \
                 tc.tile_pool(name='ops', bufs=2, space='PSUM') as ops, \
                 tc.tile_pool(name='att', bufs=2) as att:
                for j in range(HS):
                    kr0 = j + 6  # kv slab row of first key row (interior pattern)
                    psL = aps.tile([96, KS, 128], F32, tag='L')
                    lhs = q_ext[:, j * 96:(j + 1) * 96]
                    ke = k_ext[:].rearrange('c (r w) -> c r w', w=96)
                    nc.tensor.matmul(psL[:, 0:4, 0:96], lhs,
                                     ke[:, kr0:kr0 + 8:2, :], start=True, stop=True)
                    nc.tensor.matmul(psL[:, 4:7, 0:96], lhs,
                                     ke[:, kr0 + 8:kr0 + 14:2, :], start=True, stop=True)
                    lm = att.tile([96, KS, 96], F16, tag='lm')
                    nc.vector.tensor_tensor(out=lm[:], in0=psL[:, :, 0:96],
                                            in1=wb_b, op=mybir.AluOpType.add)
                    pexp = att.tile([96, KS, 96], F16, tag='p')
                    nc.scalar.activation(out=pexp[:], in_=lm[:],
                                         func=mybir.ActivationFunctionType.Exp,
                                         accum_out=den_all[:, j:j + 1])
                    psT = tp2.tile([96, KS, 128], F16, tag='T')
                    for i in range(KS):
                        nc.tensor.transpose(psT[:, i, 0:96], pexp[:, i, :],
                                            ident[0:96, 0:96])
                    pt = att.tile([96, KS, 96], F16, tag='pt')
                    if j % 2 == 0:
                        nc.scalar.copy(pt[:], psT[:, :, 0:96])
                    else:
                        nc.vector.tensor_copy(out=pt[:], in_=psT[:, :, 0:96])
                    psO = ops.tile([128, 96], F32, tag='O')
                    for i in range(KS):
                        r = kr0 + 2 * i
                        nc.tensor.matmul(psO[:], vt[:, r * 128:(r + 1) * 128],
                                         pt[:, i, :],
                                         start=(i == 0), stop=(i == KS - 1))
                    oh = att.tile([128, 96], F32, tag='oh')
                    if j % 2 == 0:
                        nc.vector.tensor_copy(out=oh[:], in_=psO[:])
                    else:
                        nc.scalar.copy(oh[:], psO[:])
                    nc.sync.dma_start(out=o[j], in_=oh[:])
                nc.sync.dma_start(out=dn[:], in_=den_all[:])

    _split_excess_waits(nc)
    _CACHE['nc'] = nc
    return nc


# ---------------------------------------------------------------- kernel ---
def _make_in_maps(x, wq, bq, wk, bk, wv, bv):
    x = np.asarray(x, dtype=np.float32)
    wq = np.asarray(wq, dtype=np.float64)
    wk = np.asarray(wk, dtype=np.float64)
    wv = np.asarray(wv, dtype=np.float64)
    bq = np.asarray(bq, dtype=np.float32)
    bk = np.asarray(bk, dtype=np.float32)
    bv = np.asarray(bv, dtype=np.float32)
    QD, KD = _hdist_channels()
    wbias = _wbias().astype(np.float16)
    wq_s = wq * SCALE

    def pack(wgt_q, wgt_k):
        # wgt_q/wgt_k: (64out, 64in, 3, 3) -> w2 (3,128,128), wa (128,128),
        # wc (64,128) with out = [q|k] (or full 128 for v)
        nout = wgt_q.shape[0] + (wgt_k.shape[0] if wgt_k is not None else 0)
        w2 = np.zeros((3, 128, 128), dtype=np.float64)
        wa = np.zeros((128, 128), dtype=np.float64)
        wc = np.zeros((64, 128), dtype=np.float64)
        for kx in range(3):
            if wgt_k is not None:
                w2[kx, 0:64, 0:64] = wgt_q[:, :, 0, kx].T
                w2[kx, 0:64, 64:128] = wgt_k[:, :, 0, kx].T
                w2[kx, 64:128, 0:64] = wgt_q[:, :, 1, kx].T
                w2[kx, 64:128, 64:128] = wgt_k[:, :, 1, kx].T
            else:
                w2[kx, 0:64, :] = wgt_q[:, :, 0, kx].T
                w2[kx, 64:128, :] = wgt_q[:, :, 1, kx].T
        if wgt_k is not None:
            wa[0:64, 0:64] = wgt_q[:, :, 2, 0].T
            wa[0:64, 64:128] = wgt_k[:, :, 2, 0].T
            wa[64:128, 0:64] = wgt_q[:, :, 2, 1].T
            wa[64:128, 64:128] = wgt_k[:, :, 2, 1].T
            wc[:, 0:64] = wgt_q[:, :, 2, 2].T
            wc[:, 64:128] = wgt_k[:, :, 2, 2].T
        else:
            wa[0:64, :] = wgt_q[:, :, 2, 0].T
            wa[64:128, :] = wgt_q[:, :, 2, 1].T
            wc[:, :] = wgt_q[:, :, 2, 2].T
        return (w2.astype(np.float16), wa.astype(np.float16),
                wc.astype(np.float16))

    qk2, qka, qkc = pack(wq_s, wk)
    v2, va, vc = pack(wv, None)
    qkbias = np.concatenate([bq * SCALE, bk]).reshape(128, 1).astype(np.float32)
    vbias = bv.reshape(128, 1).astype(np.float32)

    in_maps = []
    for core in range(NCORES):
        b, slab = core // NH, core % NH
        h0 = slab * HS
        xsl = np.zeros((64, XR, 98), dtype=np.float32)
        r_lo, r_hi = h0 - 13, h0 + 37  # image rows of slab
        src_lo, src_hi = max(0, r_lo), min(H, r_hi)
        xsl[:, src_lo - r_lo: src_hi - r_lo, 1:97] = x[b, :, src_lo:src_hi, :]
        xd = np.zeros((128, XR, 98), dtype=np.float16)
        xd[0:64] = xsl
        xd[64:128, 0:XR - 1, :] = xsl[:, 1:XR, :]
        x3d = np.zeros((128, XR, 97), dtype=np.float16)
        x3d[0:64] = xsl[:, :, 0:97]
        x3d[64:128] = xsl[:, :, 1:98]
        qdf = np.repeat(QD[:, h0:h0 + HS, None], 96, axis=2).reshape(NDIST, -1)
        kdf = np.zeros((NDIST, KV, 96), dtype=np.float64)
        for r in range(KV):
            img = h0 - 12 + r
            kdf[:, r, :] = KD[:, img % H, None]  # out-of-range rows are masked
        in_maps.append({
            'xs': xd, 'x3s': x3d,
            'wqk2': qk2, 'wqka': qka, 'wqkc': qkc,
            'wv2': v2, 'wva': va, 'wvc': vc,
            'qkb': qkbias, 'vb': vbias,
            'qd': np.ascontiguousarray(qdf, dtype=np.float16),
            'kd': np.ascontiguousarray(kdf.reshape(NDIST, -1), dtype=np.float16),
            'wbt': wbias,
        })
    return in_maps


def kernel(x, wq, bq, wk, bk, wv, bv):
    x = np.asarray(x, dtype=np.float32)
    wq = np.asarray(wq, dtype=np.float32)
    wk = np.asarray(wk, dtype=np.float32)
    wv = np.asarray(wv, dtype=np.float32)
    bq = np.asarray(bq, dtype=np.float32)
    bk = np.asarray(bk, dtype=np.float32)
    bv = np.asarray(bv, dtype=np.float32)

    nc = _build_program()
    in_maps = _make_in_maps(x=x, wq=wq, bq=bq, wk=wk, bk=bk, wv=wv, bv=bv)

    from concourse.bass_utils import run_bass_kernel_spmd
    res = run_bass_kernel_spmd(nc, in_maps, core_ids=list(range(NCORES)))
    globals()['_LAST_RES'] = res

    out = np.zeros((B, H, W, CO), dtype=np.float32)
    for core in range(NCORES):
        b, slab = core // NH, core % NH
        ot = res.results[core]['o']          # (HS, 128, 96) unnormalized O^T
        dd = res.results[core]['dn']         # (96, HS)
        out[b, slab * HS:(slab + 1) * HS] = (
            ot.transpose(0, 2, 1) / dd.T[:, :, None])

    border = _host_border(x, wq, bq, wk, bk, wv, bv)
    for h, val in border.items():
        out[:, h] = val.astype(np.float32)
    return out


# revision 8
# speedup vs baseline: 1.5430x; 1.0683x over previous
"""ConvNAT (conv QKV + 2D dilated neighborhood attention) on 8 trn2 cores.

Sharding: core = (batch b, H-slab of 24 rows).  Each core computes conv
q/k/v for its slab (+12-row halo recompute) and the attention for its 24
output rows.  H-border rows (h<6, h>=90), whose NATTEN windows are clamped
and would break SPMD program uniformity, are computed on the host in numpy
and overwrite the device output.

All matmul-path data is fp16 (psum accumulation fp32).  Conv packs the
ky=0,1 taps via a host-doubled x (row-shifted partition halves) and the
ky=2 kx=0,1 taps via a host-built column-shifted x copy: 5 matmuls per
4-row group.

Attention per output row j (transposed-logits formulation):
  logitsT[kc, i, w] = K_i^T Q_j   (7 fp16 matmuls, stationary = K row)
  expT = exp(logitsT - 4)         (ACT, reads PSUM directly)
  P^T  = expT * exp(wbias^T)      (Pool engine, fp16; mask rides as *0)
  O^T[c, w] = sum_i V_i^T P_i^T   (14 fp16 matmuls; V^T carries a ones
    row so the softmax denominator accumulates in the same psum)
  Unnormalized O^T and den are DMAed out; the host divides.
No P transposes, no psum->sbuf P copy, no on-device normalization.
"""
import os
import re
import sys

sys.path.insert(0, '/opt/trn_rl_repo')

import numpy as np

import concourse.bass as bass
from concourse import mybir
from concourse.tile import TileContext
from concourse.masks import make_identity
from bass_rust import ScopedClock, VectorClock

F32 = mybir.dt.float32
F16 = mybir.dt.float16

B, CIN, H, W = 2, 64, 96, 96
CI, CO = 64, 128
KS, DIL = 7, 2
SCALE = float(CI * 2) ** -0.5  # Cqk = 128 after pe concat
HS = 24          # rows per core
NH = 4           # h-slabs
NCORES = 8
NDIST = 7        # h-distance channels
CH = 64 + NDIST  # contraction channels
KV = 48          # k/v rows per core (24 + 12 halo each side, unclamped)
XR = 50          # x slab rows (KV + conv halo)
NEG = -30000.0
CEXP = 4.0       # constant subtracted inside exp (cancels in the ratio)
KOF = HS * 96    # k offset inside the combined qk_ext tile

# ---------------------------------------------------------------- compat ---
MAX_WAITS = 1


def _patched_drain(self, tick_clock, wait_clock):
    nc = self.nc
    ticks = [int(v) for v in re.findall(r'\d+', repr(tick_clock.global_clock))]
    for i in range(0, len(ticks), MAX_WAITS):
        chunk = [0] * len(ticks)
        chunk[i:i + MAX_WAITS] = ticks[i:i + MAX_WAITS]
        if any(chunk):
            probe = nc.sync.nop()
            wait_clock.add_sem_waits(probe.ins, ScopedClock({None: VectorClock(chunk)}))
    nc.sync.drain()
    nc.all_engine_barrier()
    popped = nc._tile_sem_poison_stack.pop()
    assert popped is self._sem_poison
    nc.clear_and_free_semaphores(list(self.sems.allocated().values()))
    nc.all_engine_barrier()


TileContext._drain_and_barrier = _patched_drain


def _split_excess_waits(nc, max_waits=MAX_WAITS):
    n_split = 0
    for fn in nc.m.functions:
        for bb in fn.blocks:
            out = []
            changed = False
            for inst in bb.instructions:
                si = inst.sync_info
                waits = list(si.on_wait) if si and si.on_wait else []
                if len(waits) > max_waits:
                    extra = waits[:-max_waits]
                    for j in range(0, len(extra), max_waits):
                        nop = mybir.InstNoOp(name=f"{inst.name}-ws{j}", ins=[], outs=[])
                        nop.engine = inst.engine
                        nop.sync_info = mybir.SyncInfo(
                            on_wait=extra[j:j + max_waits], on_update=[])
                        out.append(nop)
                    si.on_wait = waits[-max_waits:]
                    changed = True
                    n_split += 1
                out.append(inst)
            if changed:
                bb.instructions = out
    return n_split


# ------------------------------------------------------------- host math ---
def _sincos(length, dim):
    half = dim // 2
    inv_freq = 1.0 / (10000.0 ** (np.arange(half, dtype=np.float64) * 2.0 / dim))
    ang = np.arange(length, dtype=np.float64)[:, None] * inv_freq[None, :]
    return np.concatenate([np.sin(ang), np.cos(ang)], axis=-1)  # (L, dim)


def _na_indices(L, K, D):
    i = np.arange(L)
    g = i % D
    r = i // D
    Lg = (L - g + D - 1) // D
    start = np.clip(r - K // 2, 0, Lg - K)
    return g[:, None] + (start[:, None] + np.arange(K)[None, :]) * D  # (L, K)


def _hdist_channels():
    """QD,KD (NDIST, 96): sum_m QD[m,h]*KD[m,h'] == SCALE*pe_h[h].pe_h[h']
    exactly for even |h-h'| <= 6.  Magnitude-balanced per channel pair so
    fp16 rounding error stays small."""
    pe = _sincos(H, 32)           # (96, 32)
    inv_freq = 1.0 / (10000.0 ** (np.arange(16, dtype=np.float64) * 2.0 / 32))
    dv = np.array([0., 2., 4., 6.])
    g = SCALE * np.cos(dv[:, None] * inv_freq[None, :]).sum(1)  # exact pe.pe(d)
    th = np.arange(4, dtype=np.float64) * (np.pi / 6.0)
    M = np.cos(dv[:, None] * th[None, :])                        # (4, 4)
    b = np.linalg.solve(M, g)
    hh = np.arange(H, dtype=np.float64)
    QD = np.zeros((NDIST, H))
    KD = np.zeros((NDIST, H))
    QD[0] = b[0]
    KD[0] = 1.0
    for m in range(1, 4):
        QD[2 * m - 1] = b[m] * np.cos(th[m] * hh)
        QD[2 * m] = b[m] * np.sin(th[m] * hh)
        KD[2 * m - 1] = np.cos(th[m] * hh)
        KD[2 * m] = np.sin(th[m] * hh)
    for m in range(NDIST):
        mq = np.abs(QD[m]).max()
        mk = np.abs(KD[m]).max()
        if mq > 0 and mk > 0:
            s = np.sqrt(mk / mq)
            QD[m] *= s
            KD[m] /= s
    got = QD.T @ KD
    pe_ref = SCALE * (pe @ pe.T)
    for dd in (-6, -4, -2, 0, 2, 4, 6):
        idx = np.arange(max(0, -dd), min(H, H - dd))
        err = np.abs(got[idx, idx + dd] - pe_ref[idx, idx + dd]).max()
        assert err < 1e-6, (dd, err)
    return QD, KD


def _ewbias_T():
    """exp(wbias)^T (kc, w): multiplicative softmax bias; masked entries 0."""
    pe = _sincos(W, 32)
    idx_w = _na_indices(W, KS, DIL)   # (96, 7)
    wb = np.full((W, W), NEG, dtype=np.float64)
    dot = SCALE * (pe @ pe.T)
    for w in range(W):
        wb[w, idx_w[w]] = dot[w, idx_w[w]]
    return np.exp(wb.T).astype(np.float16)   # (kc, w)


def _conv_np(x, w, bias, rows):
    """NCHW 3x3 pad-1 conv evaluated at `rows` -> (B, len(rows), 96, Cout)."""
    Bn, Cin, Hn, Wn = x.shape
    xp = np.zeros((Bn, Cin, Hn + 2, Wn + 2), dtype=np.float64)
    xp[:, :, 1:-1, 1:-1] = x
    rows = np.asarray(rows)
    acc = np.zeros((Bn, len(rows), Wn, w.shape[0]), dtype=np.float64)
    for ky in range(3):
        for kx in range(3):
            xs = xp[:, :, rows + ky, :][:, :, :, kx:kx + Wn]  # (B,C,R,W)
            acc += np.einsum('bcrw,oc->brwo', xs, w[:, :, ky, kx].astype(np.float64))
    return acc + bias[None, None, None, :].astype(np.float64)


def _host_border(x, wq, bq, wk, bk, wv, bv):
    """Reference computation for the clamped border rows. -> dict h -> (B,96,128)."""
    border_h = list(range(0, 6)) + list(range(90, 96))
    kv_rows = sorted(set(np.concatenate([_na_indices(H, KS, DIL)[h] for h in border_h])))
    kv_rows = np.asarray(kv_rows)
    q_c = _conv_np(x, wq, bq, np.asarray(border_h))     # (B, 12, 96, 64)
    k_c = _conv_np(x, wk, bk, kv_rows)                  # (B, R, 96, 64)
    v_c = _conv_np(x, wv, bv, kv_rows)                  # (B, R, 96, 128)
    kv_pos = {r: i for i, r in enumerate(kv_rows)}
    pe_h = _sincos(H, 32)
    pe_w = _sincos(W, 32)
    idx_h = _na_indices(H, KS, DIL)
    idx_w = _na_indices(W, KS, DIL)
    out = {}
    for bi, h in enumerate(border_h):
        pe_q = np.concatenate([np.repeat(pe_h[h][None], W, 0), pe_w], axis=1)  # (96,64)
        q = np.concatenate([q_c[:, bi], np.repeat(pe_q[None], B, 0)], axis=2)  # (B,96,128)
        rows = [kv_pos[r] for r in idx_h[h]]
        kk = k_c[:, rows]                                   # (B,7,96,64)
        vv = v_c[:, rows]                                   # (B,7,96,128)
        pe_k = np.concatenate(
            [np.repeat(pe_h[idx_h[h]][:, None, :], W, 1),
             np.repeat(pe_w[None], KS, 0)], axis=2)         # (7,96,64)
        kk = np.concatenate([kk, np.repeat(pe_k[None], B, 0)], axis=3)  # (B,7,96,128)
        kn = kk[:, :, idx_w]                                # (B,7,96,7,128)
        vn = vv[:, :, idx_w]
        logits = SCALE * np.einsum('bwc,biwjc->bwij', q, kn)   # (B,96,7,7)
        m = logits.reshape(B, W, -1).max(-1)
        p = np.exp(logits - m[:, :, None, None])
        p /= p.reshape(B, W, -1).sum(-1)[:, :, None, None]
        out[h] = np.einsum('bwij,biwjc->bwc', p, vn)        # (B,96,128)
    return out


# ------------------------------------------------------------ bass build ---
_CACHE = {}


def _build_program():
    if 'nc' in _CACHE:
        return _CACHE['nc']
    nc = bass.Bass('TRN2')
    # x, row-doubled: [0:64]=rows, [64:128]=rows shifted +1 (for ky=0,1)
    xs = nc.dram_tensor('xs', (128, XR, 98), F16, kind='ExternalInput')
    # x, col-doubled: [0:64]=cols+0, [64:128]=cols+1 (for ky=2, kx=0,1)
    x3s = nc.dram_tensor('x3s', (128, XR, 97), F16, kind='ExternalInput')
    # all conv weights: slots 0-2 qk ky01 kx*, 3 qk ky2 kx01, 4 [qk;--] ky2kx2,
    # 5-7 v ky01 kx*, 8 v ky2 kx01, 9 [v;--] ky2kx2
    wall = nc.dram_tensor('wall', (128, 10, 128), F16, kind='ExternalInput')
    bia = nc.dram_tensor('bia', (128, 2), F32, kind='ExternalInput')
    qkd = nc.dram_tensor('qkd', (NDIST, (HS + KV) * 96), F16, kind='ExternalInput')
    ewb = nc.dram_tensor('ewb', (96, 96), F16, kind='ExternalInput')
    o = nc.dram_tensor('o', (HS // 2, 128, 2, 96), F32, kind='ExternalOutput')
    dn = nc.dram_tensor('dn', (1, HS * 96), F32, kind='ExternalOutput')

    with TileContext(nc) as tc:
        with tc.tile_pool(name='persist', bufs=1) as pp:
            # small inputs on the scalar queue; x chunks on sync in parallel
            wt = pp.tile([128, 10, 128], F16)
            nc.scalar.dma_start(out=wt, in_=wall[:])
            bias2 = pp.tile([128, 2], F32)
            nc.scalar.dma_start(out=bias2, in_=bia[:])
            ewbt = pp.tile([96, 96], F16)
            nc.scalar.dma_start(out=ewbt, in_=ewb[:])
            qk_ext = pp.tile([CH, (HS + KV) * 96], F16)
            nc.scalar.dma_start(out=qk_ext[64:CH, :], in_=qkd[:])

            x2 = pp.tile([128, XR, 98], F16)
            x3 = pp.tile([128, XR, 97], F16)
            for a, b_ in ((0, 13), (13, 26), (26, 39), (39, XR)):
                nc.sync.dma_start(out=x2[:, a:b_, :], in_=xs[:, a:b_, :])
                nc.sync.dma_start(out=x3[:, a:b_, :], in_=x3s[:, a:b_, :])

            ident = pp.tile([128, 128], F16)
            make_identity(nc, ident)
            vsb = pp.tile([128, KV * 96], F16)
            vta = pp.tile([96, KV, 65], F16)   # V^T ch 0:64 + ones col
            vtb = pp.tile([96, KV, 64], F16)   # V^T ch 64:128
            nc.gpsimd.memset(vta[:, :, 64:65], 1.0)
            den_all = pp.tile([1, HS * 96], F32)
            negc = pp.tile([96, 1], F32)
            nc.gpsimd.memset(negc, -CEXP)

            # ------------------------------------------------ convolution --
            # kv slab rows 0..47 = image rows h0-12 .. h0+35 (zero-padded x).
            # conv for kv row r uses slab rows r..r+2 (ky=0..2): ky=0,1 via
            # row-doubled x (3 matmuls, kx=0..2); ky=2 kx=0,1 via col-doubled
            # x3 (1 matmul); ky=2 kx=2 via K=64 matmul on low partitions.
            qoff = 12  # q rows are kv rows 12..35  (h0 .. h0+23)
            with tc.tile_pool(name='cps', bufs=6, space='PSUM') as cps:
                for rg in range(12):
                    r0 = rg * 4
                    for which in range(2):  # 0 = qk, 1 = v
                        s0 = 5 * which
                        t_c = cps.tile([128, 4, 96], F32, tag='c')
                        for kx in range(3):
                            nc.tensor.matmul(
                                t_c[:], wt[:, s0 + kx, :],
                                x2[:, r0:r0 + 4, kx:kx + 96],
                                start=(kx == 0), stop=False)
                        nc.tensor.matmul(
                            t_c[:], wt[:, s0 + 3, :], x3[:, r0 + 2:r0 + 6, 0:96],
                            start=False, stop=False)
                        nc.tensor.matmul(
                            t_c[:], wt[0:64, s0 + 4, :],
                            x2[0:64, r0 + 2:r0 + 6, 2:98],
                            start=False, stop=True)
                        if which == 0:
                            if qoff <= r0 < qoff + HS:
                                nc.scalar.activation(
                                    out=qk_ext[0:64, (r0 - qoff) * 96:(r0 - qoff + 4) * 96],
                                    in_=t_c[0:64], func=mybir.ActivationFunctionType.Identity,
                                    bias=bias2[0:64, 0:1])
                            nc.vector.tensor_scalar(
                                out=qk_ext[0:64, KOF + r0 * 96:KOF + (r0 + 4) * 96],
                                in0=t_c[64:128], scalar1=bias2[64:128, 0:1],
                                scalar2=None, op0=mybir.AluOpType.add)
                        else:
                            if rg % 2 == 0:
                                nc.scalar.activation(
                                    out=vsb[:, r0 * 96:(r0 + 4) * 96],
                                    in_=t_c[:], func=mybir.ActivationFunctionType.Identity,
                                    bias=bias2[:, 1:2])
                            else:
                                nc.vector.tensor_scalar(
                                    out=vsb[:, r0 * 96:(r0 + 4) * 96],
                                    in0=t_c[:], scalar1=bias2[:, 1:2],
                                    scalar2=None, op0=mybir.AluOpType.add)

            # ------------------------------------------------- V^T --------
            with tc.tile_pool(name='tps', bufs=4, space='PSUM') as tps:
                for grp in range(12):
                    pst = tps.tile([96, 4, 128], F16, tag='t')
                    for rr in range(4):
                        r = grp * 4 + rr
                        nc.tensor.transpose(
                            pst[:, rr, :], vsb[:, r * 96:(r + 1) * 96], ident)
                    if grp % 2 == 0:
                        nc.scalar.copy(vta[:, grp * 4:(grp + 1) * 4, 0:64],
                                       pst[:, :, 0:64])
                        nc.vector.tensor_copy(
                            out=vtb[:, grp * 4:(grp + 1) * 4, :], in_=pst[:, :, 64:128])
                    else:
                        nc.vector.tensor_copy(
                            out=vta[:, grp * 4:(grp + 1) * 4, 0:64], in_=pst[:, :, 0:64])
                        nc.scalar.copy(vtb[:, grp * 4:(grp + 1) * 4, :],
                                       pst[:, :, 64:128])

            # ---------------------------------------------- attention -----
            ewb_ap = ewbt[:, :]
            ewb_b = bass.AP(tensor=ewb_ap.tensor, offset=ewb_ap.offset,
                            ap=[ewb_ap.ap[0], [0, KS], ewb_ap.ap[1]])
            with tc.tile_pool(name='aps', bufs=2, space='PSUM') as aps, \
                 tc.tile_pool(name='opsa', bufs=2, space='PSUM') as opsa, \
                 tc.tile_pool(name='opsb', bufs=2, space='PSUM') as opsb, \
                 tc.tile_pool(name='att', bufs=2) as att:
                prev = None

                def emit_av(j, pex, oh2):
                    kr0 = j + 6
                    psa = opsa.tile([65, 96], F32, tag='a')
                    psb = opsb.tile([64, 96], F32, tag='b')
                    for i in range(KS):
                        r = kr0 + 2 * i
                        nc.tensor.matmul(psa[:], vta[:, r, :], pex[:, i, :],
                                         start=(i == 0), stop=(i == KS - 1))
                    for i in range(KS):
                        r = kr0 + 2 * i
                        nc.tensor.matmul(psb[:], vtb[:, r, :], pex[:, i, :],
                                         start=(i == 0), stop=(i == KS - 1))
                    jj = j % 2
                    if jj == 0:
                        nc.scalar.copy(oh2[0:64, jj, :], psa[0:64, :])
                        nc.vector.tensor_copy(out=oh2[64:128, jj, :], in_=psb[:])
                    else:
                        nc.vector.tensor_copy(out=oh2[0:64, jj, :], in_=psa[0:64, :])
                        nc.scalar.copy(oh2[64:128, jj, :], psb[:])
                    nc.vector.tensor_copy(
                        out=den_all[:, j * 96:(j + 1) * 96], in_=psa[64:65, :])
                    if jj == 1:
                        nc.sync.dma_start(out=o[j // 2], in_=oh2[:])

                oh2 = None
                for j in range(HS):
                    kr0 = j + 6  # kv slab row of first key row (interior pattern)
                    psL = aps.tile([96, KS, 128], F32, tag='L')
                    rhs = qk_ext[:, j * 96:(j + 1) * 96]
                    for i in range(KS):
                        r = kr0 + 2 * i
                        nc.tensor.matmul(
                            psL[:, i, 0:96],
                            qk_ext[:, KOF + r * 96:KOF + (r + 1) * 96], rhs,
                            start=True, stop=True)
                    ex0 = att.tile([96, KS, 96], F16, tag='e')
                    nc.scalar.activation(out=ex0[:], in_=psL[:, :, 0:96],
                                         func=mybir.ActivationFunctionType.Exp,
                                         bias=negc[:])
                    pex = att.tile([96, KS, 96], F16, tag='p')
                    nc.gpsimd.tensor_tensor(out=pex[:], in0=ex0[:], in1=ewb_b,
                                            op=mybir.AluOpType.mult)
                    if j % 2 == 0:
                        oh2 = att.tile([128, 2, 96], F32, tag='oh')
                    if prev is not None:
                        emit_av(prev, prev_pex, prev_oh2)
                    prev, prev_pex, prev_oh2 = j, pex, oh2
                emit_av(prev, prev_pex, prev_oh2)
                nc.sync.dma_start(out=dn[:], in_=den_all[:])

    _split_excess_waits(nc)
    _CACHE['nc'] = nc
    return nc


# ---------------------------------------------------------------- kernel ---
def _make_in_maps(x, wq, bq, wk, bk, wv, bv):
    x = np.asarray(x, dtype=np.float32)
    wq = np.asarray(wq, dtype=np.float64)
    wk = np.asarray(wk, dtype=np.float64)
    wv = np.asarray(wv, dtype=np.float64)
    bq = np.asarray(bq, dtype=np.float32)
    bk = np.asarray(bk, dtype=np.float32)
    bv = np.asarray(bv, dtype=np.float32)
    QD, KD = _hdist_channels()
    ewbias = _ewbias_T()
    wq_s = wq * SCALE

    wall = np.zeros((128, 10, 128), dtype=np.float64)
    for kx in range(3):
        wall[0:64, kx, 0:64] = wq_s[:, :, 0, kx].T
        wall[0:64, kx, 64:128] = wk[:, :, 0, kx].T
        wall[64:128, kx, 0:64] = wq_s[:, :, 1, kx].T
        wall[64:128, kx, 64:128] = wk[:, :, 1, kx].T
        wall[0:64, 5 + kx, :] = wv[:, :, 0, kx].T
        wall[64:128, 5 + kx, :] = wv[:, :, 1, kx].T
    wall[0:64, 3, 0:64] = wq_s[:, :, 2, 0].T
    wall[0:64, 3, 64:128] = wk[:, :, 2, 0].T
    wall[64:128, 3, 0:64] = wq_s[:, :, 2, 1].T
    wall[64:128, 3, 64:128] = wk[:, :, 2, 1].T
    wall[0:64, 4, 0:64] = wq_s[:, :, 2, 2].T
    wall[0:64, 4, 64:128] = wk[:, :, 2, 2].T
    wall[0:64, 8, :] = wv[:, :, 2, 0].T
    wall[64:128, 8, :] = wv[:, :, 2, 1].T
    wall[0:64, 9, :] = wv[:, :, 2, 2].T
    wall = wall.astype(np.float16)
    bias2 = np.stack([np.concatenate([bq * SCALE, bk]),
                      bv], axis=1).astype(np.float32)  # (128, 2)

    in_maps = []
    for core in range(NCORES):
        b, slab = core // NH, core % NH
        h0 = slab * HS
        xsl = np.zeros((64, XR, 98), dtype=np.float32)
        r_lo, r_hi = h0 - 13, h0 + 37  # image rows of slab
        src_lo, src_hi = max(0, r_lo), min(H, r_hi)
        xsl[:, src_lo - r_lo: src_hi - r_lo, 1:97] = x[b, :, src_lo:src_hi, :]
        xd = np.zeros((128, XR, 98), dtype=np.float16)
        xd[0:64] = xsl
        xd[64:128, 0:XR - 1, :] = xsl[:, 1:XR, :]
        x3d = np.zeros((128, XR, 97), dtype=np.float16)
        x3d[0:64] = xsl[:, :, 0:97]
        x3d[64:128] = xsl[:, :, 1:98]
        qdf = np.repeat(QD[:, h0:h0 + HS, None], 96, axis=2).reshape(NDIST, -1)
        kdf = np.zeros((NDIST, KV, 96), dtype=np.float64)
        for r in range(KV):
            img = h0 - 12 + r
            kdf[:, r, :] = KD[:, img % H, None]  # out-of-range rows are masked
        qkdf = np.concatenate([qdf, kdf.reshape(NDIST, -1)], axis=1)
        in_maps.append({
            'xs': xd, 'x3s': x3d, 'wall': wall, 'bia': bias2,
            'qkd': np.ascontiguousarray(qkdf, dtype=np.float16),
            'ewb': ewbias,
        })
    return in_maps


def kernel(x, wq, bq, wk, bk, wv, bv):
    x = np.asarray(x, dtype=np.float32)
    wq = np.asarray(wq, dtype=np.float32)
    wk = np.asarray(wk, dtype=np.float32)
    wv = np.asarray(wv, dtype=np.float32)
    bq = np.asarray(bq, dtype=np.float32)
    bk = np.asarray(bk, dtype=np.float32)
    bv = np.asarray(bv, dtype=np.float32)

    nc = _build_program()
    in_maps = _make_in_maps(x=x, wq=wq, bq=bq, wk=wk, bk=bk, wv=wv, bv=bv)

    from concourse.bass_utils import run_bass_kernel_spmd
    res = run_bass_kernel_spmd(nc, in_maps, core_ids=list(range(NCORES)))
    globals()['_LAST_RES'] = res

    out = np.zeros((B, H, W, CO), dtype=np.float32)
    for core in range(NCORES):
        b, slab = core // NH, core % NH
        ot = res.results[core]['o']                    # (12, 128, 2, 96)
        dd = res.results[core]['dn'].reshape(HS, 96)   # (24, 96)
        ot = ot.transpose(0, 2, 3, 1).reshape(HS, 96, CO)   # (j, w, c)
        out[b, slab * HS:(slab + 1) * HS] = ot / dd[:, :, None]

    border = _host_border(x, wq, bq, wk, bk, wv, bv)
    for h, val in border.items():
        out[:, h] = val.astype(np.float32)
    return out
